# revision 44
# baseline (speedup 1.0000x reference)
"""Distributed Trainium2 attention kernel (8 NeuronCores).

Sharding: 4-way data parallel over batch x 2-way tensor parallel over heads.
Core c handles batch c//2 and head-group c%2 (8 of 16 heads). Host sums the
two row-parallel out-proj partials per batch.

Structure (v2 — head-pair row-tiled attention):
- Phase A: per t-tile, all three projections (q,k,v) + rotary+rms for q and
  k + PE transposes into kT/qTc. PE ~5.7us/tile paces; Act does the copies/
  square/sqrt, DVE the rotary mults/reduce/reciprocal/scale.
- Phase C: heads processed in PAIRS. The two K=64 scores matmuls of a pair
  run CONCURRENTLY in the PE array via row tiling (tile_position (0,0) and
  (64,0)) — kT/qTc store the pair split at partition 64, so both tiles
  stream complementary partition ranges of the same SBUF columns (row
  tiling uses no extra XBUS). Per (pair, s-tile): 512 cy scores + 2x512 cy
  attnV = 1536 cy vs 4096 in the per-head serial schedule.
- attnV is ALSO split into K=64 key-half tiles at (0,0)/(64,0): with walrus'
  ldw-opt disabled every matmul self-loads weights, and a 64-row LDWEIGHTS
  hides under the opposite-half in-flight matmul; all-row (K=128) matmuls
  paid ~100ns exposed LDW per instruction.
- Softmax exp: one Act instruction for all of h0 (pscA) + one DVE Schraudolph
  for all of h1 (pscB). Separate psc tiles per head and separate z tiles/
  pools per writer are LOAD-BEARING: sharing a tile (or a psc read) between
  Act and DVE makes the tile-scheduler serialize the engines (its cost model
  underestimates PSUM-read ops ~2x), costing ~260us.
- attnV lags scores by 3 steps (z rings 8x), drained at each pair end so the
  next pair's evacuation closures (slots 0-6) free the single-buffered ya
  banks before attnV'(0) needs them (pair-boundary HAM trips otherwise).
- PSUM: pscA+pscB (2x2 banks double-buffered) + yaA0/yaB0/yaA1/yaB1 = 8.
- Softmax denominators ride as psum row 64 (ones column in vaug); per-pair
  normalize staged in small closures across the next pair's s-loop.
- Out-projection as a tail loop (PE has no idle in phase C); early po units
  only need early chunks' yTn so the final normalize drain hides under it.
"""
import sys
import os
from contextlib import ExitStack

if '/opt/trn_rl_repo' not in sys.path:
    sys.path.insert(0, '/opt/trn_rl_repo')

import numpy as np
import ml_dtypes

bf16 = ml_dtypes.bfloat16

T = 4096
D = 1024
HL = 8          # local heads per core
HD = 64
NT = T // 128   # 32 t-tiles
KT = D // 128   # 8 contraction tiles for projections
CW = 512        # chunk width (query columns per pair-step)
NCH = T // CW   # 8 chunks
PAIRS = 4       # head pairs per core
EPS = 1.1920928955078125e-07

EXPA = 512      # cols of the [128,1024] pair-psc on Act (true Exp = all of
                # h0); DVE Schraudolph covers all of h1. Separate z tiles so
                # Act and DVE never co-write one tile (no WAW serialization).
LN2 = 0.6931471805599453
# z = bitcast_bf16(int16(psc * SCHRA + SCHRB)) ~= exp(0.125 * psc) * const
SCHRA = 0.125 * (2.0 ** 23 / LN2) / 65536.0
SCHRB = (127.0 * 2.0 ** 23 - 485000.0) / 65536.0


def build():
    from concourse import bacc, tile, mybir

    BF16 = mybir.dt.bfloat16
    F32 = mybir.dt.float32
    I16 = mybir.dt.int16
    AF = mybir.ActivationFunctionType
    ALU = mybir.AluOpType
    AX = mybir.AxisListType

    nc = bacc.Bacc()
    xT = nc.declare_dram_parameter("xT", [D, T], BF16, isOutput=False)
    wqT = nc.declare_dram_parameter("wqT", [D, 512], BF16, isOutput=False)
    wkT = nc.declare_dram_parameter("wkT", [D, 512], BF16, isOutput=False)
    wvT = nc.declare_dram_parameter("wvT", [D, 512], BF16, isOutput=False)
    woT = nc.declare_dram_parameter("woT", [512, D], BF16, isOutput=False)
    cos2 = nc.declare_dram_parameter("cos2", [T, 64], BF16, isOutput=False)
    ss = nc.declare_dram_parameter("ss", [T, 64], BF16, isOutput=False)
    ident = nc.declare_dram_parameter("ident", [128, 128], BF16, isOutput=False)
    out = nc.declare_dram_parameter("out", [T, D], F32, isOutput=True)

    with tile.TileContext(nc) as tc:
        with tc.tile_pool(name="persist", bufs=1) as persist:
            qTc = [persist.tile([128, PAIRS, CW], BF16, tag=f"qT{c}",
                                name=f"qT{c}") for c in range(NCH)]
            kT = persist.tile([128, PAIRS, T], BF16, tag="kT")
            vaug = persist.tile([128, NT, HL, 65], BF16, tag="vaug")
            wo_sb = persist.tile([128, 4, D], BF16, tag="wo_sb")
            id_sb = persist.tile([128, 128], BF16, tag="id_sb")
            eps_t = persist.tile([128, 1], F32, tag="eps_t")
            yTn = persist.tile([128, PAIRS, T], BF16, tag="yTn")

            nc.vector.memset(vaug[:, :, :, 64:65], 1.0)
            nc.vector.memset(eps_t[:], EPS)

            # ================= Phase A: q/k/v for all t =================
            with ExitStack() as phaseA:
                wkv = phaseA.enter_context(tc.tile_pool(name="wkv", bufs=1))
                xcolp = phaseA.enter_context(
                    tc.tile_pool(name="xcolp", bufs=5))
                ascr = phaseA.enter_context(tc.tile_pool(name="ascr", bufs=3))
                asmall = phaseA.enter_context(
                    tc.tile_pool(name="asmall", bufs=3))
                ps_qkv = phaseA.enter_context(
                    tc.tile_pool(name="ps_qkv", bufs=2, space="PSUM"))
                ps_tr = phaseA.enter_context(
                    tc.tile_pool(name="ps_tr", bufs=2, space="PSUM"))

                def dma_xcol(t):
                    xcol = xcolp.tile([128, KT, 128], BF16, tag="xcol")
                    nc.sync.dma_start(
                        xcol[:],
                        xT[:, t * 128:(t + 1) * 128].rearrange(
                            "(k p) t -> p k t", p=128))
                    return xcol

                # DMA issue order = first-use order: tile 0 needs wk+xcol0
                # within ~5us, the rest can trail
                w_sb = {}
                for name, param in (("k", wkT), ("v", wvT), ("q", wqT)):
                    w_sb[name] = wkv.tile([128, KT, 512], BF16,
                                          tag=f"w{name}", name=f"w_{name}_sb")
                xq = [dma_xcol(0)]
                for ki in range(KT):
                    nc.sync.dma_start(
                        w_sb["k"][:, ki, :], wkT[ki * 128:(ki + 1) * 128, :])
                for ki in range(KT):
                    nc.sync.dma_start(
                        w_sb["v"][:, ki, :], wvT[ki * 128:(ki + 1) * 128, :])
                xq.append(dma_xcol(1))
                for ki in range(KT):
                    nc.sync.dma_start(
                        w_sb["q"][:, ki, :], wqT[ki * 128:(ki + 1) * 128, :])
                xq.append(dma_xcol(2))
                # cos/ss feed the rotary chains (DVE), which lag the
                # projections by design — their DMAs can land late
                cos_sb = wkv.tile([128, NT, 64], BF16, tag="cos_sb")
                ss_sb = wkv.tile([128, NT, 64], BF16, tag="ss_sb")
                nc.sync.dma_start(
                    cos_sb[:], cos2[:].rearrange("(t p) d -> p t d", p=128))
                nc.sync.dma_start(
                    ss_sb[:], ss[:].rearrange("(t p) d -> p t d", p=128))
                xq.append(dma_xcol(3))
                xq.append(dma_xcol(4))
                nc.sync.dma_start(id_sb[:], ident[:])

                # preload the GpSimd libraries (PartitionBroadcast + copy)
                # now so phase C's first use doesn't eat a Q7 reload
                pbsrc = asmall.tile([1, 8], F32, tag="pbsrc")
                nc.vector.memset(pbsrc[:], 1.0)
                pbdst = asmall.tile([128, 8], F32, tag="pbdst")
                nc.gpsimd.partition_broadcast(pbdst[:], pbsrc[:])

                def proj(xcol, name):
                    ps = ps_qkv.tile([128, 512], F32, tag=f"p{name}",
                                     name=f"ps_{name}")
                    for ki in range(KT):
                        nc.tensor.matmul(
                            ps[:], xcol[:, ki, :], w_sb[name][:, ki, :],
                            start=(ki == 0), stop=(ki == KT - 1))
                    return ps

                def rotary_rms(t, ps_q, store, tcol):
                    """rotary + rms-normalize one projected [128,512] tile.
                    Copies + square + sqrt on Act; mults/reduce/reciprocal/
                    scale on DVE. Returns the qn tile to transpose later."""
                    ctb = cos_sb[:, t, :].unsqueeze(1).broadcast_to(
                        [128, HL, 64])
                    stb = ss_sb[:, t, :].unsqueeze(1).broadcast_to(
                        [128, HL, 64])
                    qb = ascr.tile([128, 512], BF16, tag="qb")
                    nc.scalar.copy(qb[:], ps_q[:])
                    b3 = qb[:].rearrange("p (h u d) -> p h u d", h=HL, u=2)
                    qs = ascr.tile([128, 512], BF16, tag="qs")
                    qs3 = qs[:].rearrange("p (h u d) -> p h u d", h=HL, u=2)
                    nc.scalar.copy(qs3[:, :, 0, :], b3[:, :, 1, :])
                    nc.scalar.copy(qs3[:, :, 1, :], b3[:, :, 0, :])
                    t1 = ascr.tile([128, 512], BF16, tag="t1")
                    nc.vector.tensor_tensor(
                        t1[:].rearrange("p (h d) -> p h d", h=HL),
                        qb[:].rearrange("p (h d) -> p h d", h=HL),
                        ctb, op=ALU.mult)
                    r = ascr.tile([128, 512], BF16, tag="r")
                    nc.vector.tensor_tensor(
                        r[:].rearrange("p (h d) -> p h d", h=HL),
                        qs[:].rearrange("p (h d) -> p h d", h=HL),
                        stb, op=ALU.mult)
                    nc.vector.tensor_tensor(r[:], t1[:], r[:], op=ALU.add)
                    sq = ascr.tile([128, 512], BF16, tag="sq")
                    nc.scalar.square(sq[:], r[:])
                    ms8 = asmall.tile([128, HL], F32, tag="ms8")
                    nc.vector.tensor_reduce(
                        ms8[:], sq[:].rearrange("p (h d) -> p h d", h=HL),
                        axis=AX.X, op=ALU.add)
                    rms = asmall.tile([128, HL], F32, tag="rms")
                    nc.scalar.activation(
                        rms[:], ms8[:], AF.Sqrt, scale=1.0 / HD,
                        bias=eps_t[:])
                    rinv = asmall.tile([128, HL], F32, tag="rinv")
                    nc.vector.reciprocal(rinv[:], rms[:])
                    qn = ascr.tile([128, 512], BF16, tag="qn")
                    nc.vector.tensor_tensor(
                        qn[:].rearrange("p (h d) -> p h d", h=HL),
                        r[:].rearrange("p (h d) -> p h d", h=HL),
                        rinv[:].unsqueeze(2).broadcast_to([128, HL, 64]),
                        op=ALU.mult)
                    return (qn, store, tcol)

                def emit_tr(pend, on_act):
                    qn, store, tcol = pend
                    tp = ps_tr.tile([128, 4, 128], BF16, tag="tp")
                    for cb in range(4):
                        nc.tensor.transpose(
                            tp[:, cb, :], qn[:, cb * 128:(cb + 1) * 128],
                            id_sb[:])
                    cp = nc.scalar.copy if on_act else nc.vector.tensor_copy
                    cp(store[:, :, tcol:tcol + 128], tp[:])

                pendq = []
                for t in range(NT):
                    xcol = xq.pop(0)
                    if t + 5 < NT:
                        xq.append(dma_xcol(t + 5))
                    ps_k = proj(xcol, "k")
                    ps_v = proj(xcol, "v")
                    ps_q = proj(xcol, "q")
                    nc.scalar.copy(
                        vaug[:, t, :, 0:64],
                        ps_v[:].rearrange("p (h d) -> p h d", h=HL))
                    if len(pendq) >= 4:
                        emit_tr(pendq.pop(0), on_act=False)
                    pendq.append(rotary_rms(t, ps_k, kT, t * 128))
                    if len(pendq) >= 4:
                        emit_tr(pendq.pop(0), on_act=True)
                    pendq.append(
                        rotary_rms(t, ps_q, qTc[t // 4], (t % 4) * 128))
                # A->C bridge INSIDE phase A scope: Exp table load (ATL
                # ~2.7us on Act) + PE junk burst covering the last tiles'
                # DVE rotary drain, then the final transposes
                nc.sync.dma_start(
                    wo_sb[:], woT[:].rearrange("(k p) n -> p k n", p=128))
                dummy0 = asmall.tile([128, 1], F32, tag="dummy0")
                nc.scalar.activation(dummy0[:], eps_t[:], AF.Exp)
                # burst sized to the ~10us end-of-phase-A DVE rotary
                # backlog: PE chews junk at full clock while DVE drains,
                # so the drain transposes never leave a >2us PE idle
                wup = ps_qkv.tile([128, 512], F32, tag="pk", name="wup")
                for i in range(64):
                    nc.tensor.matmul(
                        wup[:], kT[0:64, 0, 0:128], kT[0:64, 0, 0:512],
                        start=(i == 0), stop=(i == 63))
                for i, pend in enumerate(pendq):
                    emit_tr(pend, on_act=(i % 2 == 1))
                pendq = []

            # ================= Phase C: attention =================
            with (
                tc.tile_pool(name="zpA", bufs=6) as zpA,
                tc.tile_pool(name="zpB", bufs=6) as zpB,
                tc.tile_pool(name="nrm", bufs=2) as nrm,
                tc.tile_pool(name="nrm1", bufs=2) as nrm1,
                tc.tile_pool(name="dtp", bufs=4) as dtp,
                tc.tile_pool(name="bcp", bufs=3) as bcp,
                tc.tile_pool(name="ps_sc", bufs=2, space="PSUM") as ps_sc,
                tc.tile_pool(name="ps_y", bufs=1, space="PSUM") as ps_y,
            ):
                def emit_attnv(e):
                    """attnV for one (pair, s): each head's matmul is split
                    into two K=64 key-half tiles at row positions (0,0) and
                    (64,0) — concurrent in the PE array, and every LDWEIGHTS
                    targets row groups disjoint from the in-flight matmul so
                    weight loads hide. Halves accumulate into separate psum
                    banks (yaA+yaB summed during normalize)."""
                    zA, zB, ya, s, pr = e
                    yaA0, yaB0, yaA1, yaB1 = ya
                    st0, st1 = (s == 0), (s == NT - 1)
                    nc.tensor.matmul(
                        yaA0[:], vaug[0:64, s, 2 * pr, :], zA[0:64, :],
                        start=st0, stop=st1, tile_position=(0, 0))
                    nc.tensor.matmul(
                        yaB0[:], vaug[64:128, s, 2 * pr, :], zA[64:128, :],
                        start=st0, stop=st1, tile_position=(64, 0))
                    nc.tensor.matmul(
                        yaA1[:], vaug[0:64, s, 2 * pr + 1, :], zB[0:64, :],
                        start=st0, stop=st1, tile_position=(0, 0))
                    nc.tensor.matmul(
                        yaB1[:], vaug[64:128, s, 2 * pr + 1, :],
                        zB[64:128, :],
                        start=st0, stop=st1, tile_position=(64, 0))

                def make_norm(ya, pr, ch):
                    """normalize pair (ch, pr): 7 small closures staged
                    across the next pair's s-loop (slots >= 3, i.e. after
                    this pair's final attnV has been emitted). Evacuation
                    merges the key-half psum banks (copy + add: only one
                    PSUM operand per DVE instruction). Row 64 of the merged
                    tile is the softmax denominator; it is re-staged to a
                    partition-0 tile for the gpsimd broadcast."""
                    yab = ((ya[0], ya[1]), (ya[2], ya[3]))
                    st = {}
                    c0 = ch * CW

                    def evac(i):
                        a, b = yab[i]
                        yu = nrm.tile([65, CW], F32, tag=f"yu{i}",
                                      name=f"yu{i}_{ch}_{pr}")
                        nc.scalar.copy(yu[:], a[:])
                        nc.vector.tensor_tensor(
                            yu[:], yu[:], b[:], op=ALU.add)
                        st[f"yu{i}"] = yu

                    def bc(i):
                        # denominator row broadcast via SBUF->SBUF DMA
                        # (0-stride partition read): the DMA engines are
                        # idle in phase C, and this keeps GpSimd and the
                        # Act queue out of the normalize chain (the
                        # Act->GpSimd WAR coupling starved the exps at
                        # pair boundaries -> 6.8us HAM half-clock dips)
                        b = bcp.tile([64, CW], F32, tag=f"bc{i}",
                                     name=f"bc{i}_{ch}_{pr}")
                        nc.sync.dma_start(
                            b[:],
                            st[f"yu{i}"][64:65, :].unsqueeze(1)
                            .broadcast_to([1, 64, CW]))
                        st[f"bc{i}"] = b

                    def recip():
                        for i in (0, 1):
                            rc = nrm1.tile([64, CW], F32, tag=f"bcr{i}",
                                           name=f"bcr{i}_{ch}_{pr}")
                            nc.vector.reciprocal_approx_fast(
                                rc[:], st[f"bc{i}"][:])
                            st[f"r{i}"] = rc

                    def mult(i):
                        # stays on DVE: gpsimd would work (all-SBUF) but
                        # switching its Q7 library away from
                        # PartitionBroadcast costs a multi-us reload per
                        # closure (measured +230us total)
                        rsl = slice(64 * i, 64 * i + 64)
                        nc.vector.tensor_tensor(
                            yTn[rsl, pr, c0:c0 + CW],
                            st[f"yu{i}"][0:64, :], st[f"r{i}"][:],
                            op=ALU.mult)

                    return [lambda: evac(0), lambda: evac(1),
                            lambda: bc(0), lambda: bc(1), recip,
                            lambda: mult(0), lambda: mult(1)]

                pend = []      # z tiles awaiting attnV (lag 3)
                norm_q = []    # previous pair's normalize closures
                for ch in range(NCH):
                    for pr in range(PAIRS):
                        ya = tuple(
                            ps_y.tile([65, CW], F32, tag=f"ya{t}",
                                      name=f"ya{t}_{ch}_{pr}")
                            for t in ("A0", "B0", "A1", "B1"))
                        for s in range(NT):
                            if norm_q and s <= 6:
                                norm_q.pop(0)()
                            ssl = slice(s * 128, (s + 1) * 128)
                            pscA = ps_sc.tile([128, CW], F32, tag="pscA")
                            pscB = ps_sc.tile([128, CW], F32, tag="pscB")
                            nc.tensor.matmul(
                                pscA[:], kT[0:64, pr, ssl],
                                qTc[ch][0:64, pr, :],
                                start=True, stop=True, tile_position=(0, 0))
                            nc.tensor.matmul(
                                pscB[:], kT[64:128, pr, ssl],
                                qTc[ch][64:128, pr, :],
                                start=True, stop=True, tile_position=(64, 0))
                            zA = zpA.tile([128, CW], BF16, tag="zA")
                            zB = zpB.tile([128, CW], BF16, tag="zB")
                            nc.scalar.activation(
                                zA[:], pscA[:], AF.Exp, scale=0.125)
                            nc.vector.tensor_scalar(
                                zB[:].bitcast(I16), pscB[:],
                                SCHRA, SCHRB, ALU.mult, ALU.add)
                            pend.append((zA, zB, ya, s, pr))
                            if len(pend) > 3:
                                emit_attnv(pend.pop(0))
                        # drain this pair's attnVs now (the last one waits
                        # ~0.3us on its exp) so the evacuations can run at
                        # the next pair's first slots — attnV'(0) then finds
                        # the ya banks free (no >2us stall, no HAM trip)
                        for e in pend:
                            emit_attnv(e)
                        pend = []
                        assert not norm_q
                        norm_q = make_norm(ya, pr, ch)
                for e in pend:
                    emit_attnv(e)
                pend = []
                for fn in norm_q:
                    fn()
                norm_q = []

            # ===== tail: out-projection for all t =====
            with (
                tc.tile_pool(name="ps_po", bufs=4, space="PSUM") as ps_po,
                tc.tile_pool(name="ostg", bufs=4) as ostg,
            ):
                # bridge the final normalize drain (DVE ~3us) so the PE
                # stays warm into the out-projection
                wdn = ps_po.tile([128, 512], F32, tag="po", name="wdn")
                for i in range(14):
                    nc.tensor.matmul(
                        wdn[:], kT[0:64, 0, 0:128], kT[0:64, 0, 0:512],
                        start=(i == 0), stop=(i == 13))
                for tt in range(NT):
                    tsl = slice(tt * 128, (tt + 1) * 128)
                    for oc in range(2):
                        po = ps_po.tile([128, 512], F32, tag="po")
                        for kp in range(4):
                            nc.tensor.matmul(
                                po[:], yTn[:, kp, tsl],
                                wo_sb[:, kp, oc * 512:(oc + 1) * 512],
                                start=(kp == 0), stop=(kp == 3))
                        ost = ostg.tile([128, 512], F32, tag="ost")
                        nc.vector.tensor_copy(ost[:], po[:])
                        nc.sync.dma_start(
                            out[tsl, oc * 512:(oc + 1) * 512], ost[:])

    nc.compile()
    return nc


_CACHE = {}


def _get_nc():
    if "nc" not in _CACHE:
        _CACHE["nc"] = build()
    return _CACHE["nc"]


def _prep_inputs(x, cos, sin, wq, wk, wv, wo):
    x = np.asarray(x, dtype=np.float32)
    cos = np.asarray(cos, dtype=np.float32).reshape(T, 32)
    sin = np.asarray(sin, dtype=np.float32).reshape(T, 32)
    wq = np.asarray(wq, dtype=np.float32)
    wk = np.asarray(wk, dtype=np.float32)
    wv = np.asarray(wv, dtype=np.float32)
    wo = np.asarray(wo, dtype=np.float32)

    cos2 = np.concatenate([cos, cos], axis=1)
    ss = np.concatenate([sin, -sin], axis=1)
    ident = np.eye(128, dtype=bf16)

    in_maps = []
    for c in range(8):
        b, hg = c // 2, c % 2
        rows = slice(hg * 512, (hg + 1) * 512)
        in_maps.append({
            "xT": np.ascontiguousarray(x[b].T).astype(bf16),
            "wqT": np.ascontiguousarray(wq[rows, :].T).astype(bf16),
            "wkT": np.ascontiguousarray(wk[rows, :].T).astype(bf16),
            "wvT": np.ascontiguousarray(wv[rows, :].T).astype(bf16),
            "woT": np.ascontiguousarray(wo[:, rows].T).astype(bf16),
            "cos2": cos2.astype(bf16),
            "ss": ss.astype(bf16),
            "ident": ident,
        })
    return in_maps


def _run(in_maps, trace=False):
    from concourse.bass_utils import run_bass_kernel_spmd

    nc = _get_nc()
    res = run_bass_kernel_spmd(nc, in_maps, core_ids=list(range(8)),
                               trace=trace)
    parts = [res.results[c]["out"] for c in range(8)]
    full = np.stack([parts[2 * b] + parts[2 * b + 1] for b in range(4)])
    return full.astype(np.float32), res


def kernel(x, cos, sin, wq, wk, wv, wo):
    in_maps = _prep_inputs(x, cos, sin, wq, wk, wv, wo)
    full, _ = _run(in_maps, trace=False)
    return full


# revision 47
# speedup vs baseline: 1.2235x; 1.2235x over previous
"""Distributed Trainium2 attention kernel (8 NeuronCores).

Sharding: 4-way data parallel over batch x 2-way tensor parallel over heads.
Core c handles batch c//2 and head-group c%2 (8 of 16 heads). Host sums the
two row-parallel out-proj partials per batch.

Structure (v2 — head-pair row-tiled attention):
- Phase A: per t-tile, all three projections (q,k,v) + rotary+rms for q and
  k + PE transposes into kT/qTc. PE ~5.7us/tile paces; Act does the copies/
  square/sqrt, DVE the rotary mults/reduce/reciprocal/scale.
- Phase C: heads processed in PAIRS. The two K=64 scores matmuls of a pair
  run CONCURRENTLY in the PE array via row tiling (tile_position (0,0) and
  (64,0)) — kT/qTc store the pair split at partition 64, so both tiles
  stream complementary partition ranges of the same SBUF columns (row
  tiling uses no extra XBUS). Per (pair, s-tile): 512 cy scores + 2x512 cy
  attnV = 1536 cy vs 4096 in the per-head serial schedule.
- attnV is ALSO split into K=64 key-half tiles at (0,0)/(64,0): with walrus'
  ldw-opt disabled every matmul self-loads weights, and a 64-row LDWEIGHTS
  hides under the opposite-half in-flight matmul; all-row (K=128) matmuls
  paid ~100ns exposed LDW per instruction.
- Softmax exp: one Act instruction for all of h0 (pscA) + one DVE Schraudolph
  for all of h1 (pscB). Separate psc tiles per head and separate z tiles/
  pools per writer are LOAD-BEARING: sharing a tile (or a psc read) between
  Act and DVE makes the tile-scheduler serialize the engines (its cost model
  underestimates PSUM-read ops ~2x), costing ~260us.
- attnV lags scores by 3 steps (z rings 8x), drained at each pair end so the
  next pair's evacuation closures (slots 0-6) free the single-buffered ya
  banks before attnV'(0) needs them (pair-boundary HAM trips otherwise).
- PSUM: pscA+pscB (2x2 banks double-buffered) + yaA0/yaB0/yaA1/yaB1 = 8.
- Softmax denominators ride as psum row 64 (ones column in vaug); per-pair
  normalize staged in small closures across the next pair's s-loop.
- Out-projection as a tail loop (PE has no idle in phase C); early po units
  only need early chunks' yTn so the final normalize drain hides under it.
"""
import sys
import os
from contextlib import ExitStack

if '/opt/trn_rl_repo' not in sys.path:
    sys.path.insert(0, '/opt/trn_rl_repo')

import numpy as np
import ml_dtypes

bf16 = ml_dtypes.bfloat16

T = 4096
D = 1024
HL = 8          # local heads per core
HD = 64
NT = T // 128   # 32 t-tiles
KT = D // 128   # 8 contraction tiles for projections
CW = 512        # chunk width (query columns per pair-step)
NCH = T // CW   # 8 chunks
PAIRS = 4       # head pairs per core
EPS = 1.1920928955078125e-07

EXPA = 512      # cols of the [128,1024] pair-psc on Act (true Exp = all of
                # h0); DVE Schraudolph covers all of h1. Separate z tiles so
                # Act and DVE never co-write one tile (no WAW serialization).
LN2 = 0.6931471805599453
# z = bitcast_bf16(int16(psc * SCHRA + SCHRB)) ~= exp(0.125 * psc) * const
SCHRA = 0.125 * (2.0 ** 23 / LN2) / 65536.0
SCHRB = (127.0 * 2.0 ** 23 - 485000.0) / 65536.0


def build():
    from concourse import bacc, tile, mybir

    BF16 = mybir.dt.bfloat16
    F32 = mybir.dt.float32
    I16 = mybir.dt.int16
    AF = mybir.ActivationFunctionType
    ALU = mybir.AluOpType
    AX = mybir.AxisListType

    nc = bacc.Bacc()
    xT = nc.declare_dram_parameter("xT", [D, T], BF16, isOutput=False)
    wqT = nc.declare_dram_parameter("wqT", [D, 512], BF16, isOutput=False)
    wkT = nc.declare_dram_parameter("wkT", [D, 512], BF16, isOutput=False)
    wvT = nc.declare_dram_parameter("wvT", [D, 512], BF16, isOutput=False)
    woT = nc.declare_dram_parameter("woT", [512, D], BF16, isOutput=False)
    cos2 = nc.declare_dram_parameter("cos2", [T, 64], BF16, isOutput=False)
    ss = nc.declare_dram_parameter("ss", [T, 64], BF16, isOutput=False)
    ident = nc.declare_dram_parameter("ident", [128, 128], BF16, isOutput=False)
    out = nc.declare_dram_parameter("out", [T, D], F32, isOutput=True)

    with tile.TileContext(nc) as tc:
        with tc.tile_pool(name="persist", bufs=1) as persist:
            qTc = [persist.tile([128, PAIRS, CW], BF16, tag=f"qT{c}",
                                name=f"qT{c}") for c in range(NCH)]
            kT = persist.tile([128, PAIRS, T], BF16, tag="kT")
            vaug = persist.tile([128, NT, HL, 65], BF16, tag="vaug")
            wo_sb = persist.tile([128, 4, D], BF16, tag="wo_sb")
            id_sb = persist.tile([128, 128], BF16, tag="id_sb")
            eps_t = persist.tile([128, 1], F32, tag="eps_t")
            yTn = persist.tile([128, PAIRS, T], BF16, tag="yTn")

            nc.vector.memset(vaug[:, :, :, 64:65], 1.0)
            nc.vector.memset(eps_t[:], EPS)

            # ================= Phase A: q/k/v for all t =================
            with ExitStack() as phaseA:
                wkv = phaseA.enter_context(tc.tile_pool(name="wkv", bufs=1))
                xcolp = phaseA.enter_context(
                    tc.tile_pool(name="xcolp", bufs=5))
                ascr = phaseA.enter_context(tc.tile_pool(name="ascr", bufs=3))
                asmall = phaseA.enter_context(
                    tc.tile_pool(name="asmall", bufs=3))
                ps_qkv = phaseA.enter_context(
                    tc.tile_pool(name="ps_qkv", bufs=2, space="PSUM"))
                ps_tr = phaseA.enter_context(
                    tc.tile_pool(name="ps_tr", bufs=2, space="PSUM"))

                def dma_xcol(t):
                    xcol = xcolp.tile([128, KT, 128], BF16, tag="xcol")
                    nc.sync.dma_start(
                        xcol[:],
                        xT[:, t * 128:(t + 1) * 128].rearrange(
                            "(k p) t -> p k t", p=128))
                    return xcol

                # DMA issue order = first-use order: tile 0 needs wk+xcol0
                # within ~5us, the rest can trail
                w_sb = {}
                for name, param in (("k", wkT), ("v", wvT), ("q", wqT)):
                    w_sb[name] = wkv.tile([128, KT, 512], BF16,
                                          tag=f"w{name}", name=f"w_{name}_sb")
                xq = [dma_xcol(0)]
                for ki in range(KT):
                    nc.sync.dma_start(
                        w_sb["k"][:, ki, :], wkT[ki * 128:(ki + 1) * 128, :])
                for ki in range(KT):
                    nc.sync.dma_start(
                        w_sb["v"][:, ki, :], wvT[ki * 128:(ki + 1) * 128, :])
                xq.append(dma_xcol(1))
                for ki in range(KT):
                    nc.sync.dma_start(
                        w_sb["q"][:, ki, :], wqT[ki * 128:(ki + 1) * 128, :])
                xq.append(dma_xcol(2))
                # cos/ss feed the rotary chains (DVE), which lag the
                # projections by design — their DMAs can land late
                cos_sb = wkv.tile([128, NT, 64], BF16, tag="cos_sb")
                ss_sb = wkv.tile([128, NT, 64], BF16, tag="ss_sb")
                nc.sync.dma_start(
                    cos_sb[:], cos2[:].rearrange("(t p) d -> p t d", p=128))
                nc.sync.dma_start(
                    ss_sb[:], ss[:].rearrange("(t p) d -> p t d", p=128))
                xq.append(dma_xcol(3))
                xq.append(dma_xcol(4))
                nc.sync.dma_start(id_sb[:], ident[:])

                # preload the GpSimd libraries (PartitionBroadcast + copy)
                # now so phase C's first use doesn't eat a Q7 reload
                pbsrc = asmall.tile([1, 8], F32, tag="pbsrc")
                nc.vector.memset(pbsrc[:], 1.0)
                pbdst = asmall.tile([128, 8], F32, tag="pbdst")
                nc.gpsimd.partition_broadcast(pbdst[:], pbsrc[:])

                def proj(xcol, name):
                    ps = ps_qkv.tile([128, 512], F32, tag=f"p{name}",
                                     name=f"ps_{name}")
                    for ki in range(KT):
                        nc.tensor.matmul(
                            ps[:], xcol[:, ki, :], w_sb[name][:, ki, :],
                            start=(ki == 0), stop=(ki == KT - 1))
                    return ps

                def rotary_rms(t, ps_q, store, tcol):
                    """rotary + rms-normalize one projected [128,512] tile.
                    Copies + square + sqrt on Act; mults/reduce/reciprocal/
                    scale on DVE. Returns the qn tile to transpose later."""
                    ctb = cos_sb[:, t, :].unsqueeze(1).broadcast_to(
                        [128, HL, 64])
                    stb = ss_sb[:, t, :].unsqueeze(1).broadcast_to(
                        [128, HL, 64])
                    qb = ascr.tile([128, 512], BF16, tag="qb")
                    nc.scalar.copy(qb[:], ps_q[:])
                    b3 = qb[:].rearrange("p (h u d) -> p h u d", h=HL, u=2)
                    qs = ascr.tile([128, 512], BF16, tag="qs")
                    qs3 = qs[:].rearrange("p (h u d) -> p h u d", h=HL, u=2)
                    nc.scalar.copy(qs3[:, :, 0, :], b3[:, :, 1, :])
                    nc.scalar.copy(qs3[:, :, 1, :], b3[:, :, 0, :])
                    t1 = ascr.tile([128, 512], BF16, tag="t1")
                    nc.vector.tensor_tensor(
                        t1[:].rearrange("p (h d) -> p h d", h=HL),
                        qb[:].rearrange("p (h d) -> p h d", h=HL),
                        ctb, op=ALU.mult)
                    r = ascr.tile([128, 512], BF16, tag="r")
                    nc.vector.tensor_tensor(
                        r[:].rearrange("p (h d) -> p h d", h=HL),
                        qs[:].rearrange("p (h d) -> p h d", h=HL),
                        stb, op=ALU.mult)
                    nc.vector.tensor_tensor(r[:], t1[:], r[:], op=ALU.add)
                    sq = ascr.tile([128, 512], BF16, tag="sq")
                    nc.scalar.square(sq[:], r[:])
                    ms8 = asmall.tile([128, HL], F32, tag="ms8")
                    nc.vector.tensor_reduce(
                        ms8[:], sq[:].rearrange("p (h d) -> p h d", h=HL),
                        axis=AX.X, op=ALU.add)
                    rms = asmall.tile([128, HL], F32, tag="rms")
                    nc.scalar.activation(
                        rms[:], ms8[:], AF.Sqrt, scale=1.0 / HD,
                        bias=eps_t[:])
                    rinv = asmall.tile([128, HL], F32, tag="rinv")
                    nc.vector.reciprocal(rinv[:], rms[:])
                    qn = ascr.tile([128, 512], BF16, tag="qn")
                    nc.vector.tensor_tensor(
                        qn[:].rearrange("p (h d) -> p h d", h=HL),
                        r[:].rearrange("p (h d) -> p h d", h=HL),
                        rinv[:].unsqueeze(2).broadcast_to([128, HL, 64]),
                        op=ALU.mult)
                    return (qn, store, tcol)

                def emit_tr(pend, on_act):
                    qn, store, tcol = pend
                    tp = ps_tr.tile([128, 4, 128], BF16, tag="tp")
                    for cb in range(4):
                        nc.tensor.transpose(
                            tp[:, cb, :], qn[:, cb * 128:(cb + 1) * 128],
                            id_sb[:])
                    cp = nc.scalar.copy if on_act else nc.vector.tensor_copy
                    cp(store[:, :, tcol:tcol + 128], tp[:])

                pendq = []
                for t in range(NT):
                    xcol = xq.pop(0)
                    if t + 5 < NT:
                        xq.append(dma_xcol(t + 5))
                    ps_k = proj(xcol, "k")
                    ps_v = proj(xcol, "v")
                    ps_q = proj(xcol, "q")
                    nc.scalar.copy(
                        vaug[:, t, :, 0:64],
                        ps_v[:].rearrange("p (h d) -> p h d", h=HL))
                    if len(pendq) >= 4:
                        emit_tr(pendq.pop(0), on_act=False)
                    pendq.append(rotary_rms(t, ps_k, kT, t * 128))
                    if len(pendq) >= 4:
                        emit_tr(pendq.pop(0), on_act=True)
                    pendq.append(
                        rotary_rms(t, ps_q, qTc[t // 4], (t % 4) * 128))
                # A->C bridge INSIDE phase A scope: Exp table load (ATL
                # ~2.7us on Act) + PE junk burst covering the last tiles'
                # DVE rotary drain, then the final transposes
                nc.sync.dma_start(
                    wo_sb[:], woT[:].rearrange("(k p) n -> p k n", p=128))
                dummy0 = asmall.tile([128, 1], F32, tag="dummy0")
                nc.scalar.activation(dummy0[:], eps_t[:], AF.Exp)
                # burst sized to the ~10us end-of-phase-A DVE rotary
                # backlog: PE chews junk at full clock while DVE drains,
                # so the drain transposes never leave a >2us PE idle
                wup = ps_qkv.tile([128, 512], F32, tag="pk", name="wup")
                for i in range(64):
                    nc.tensor.matmul(
                        wup[:], kT[0:64, 0, 0:128], kT[0:64, 0, 0:512],
                        start=(i == 0), stop=(i == 63))
                for i, pend in enumerate(pendq):
                    emit_tr(pend, on_act=(i % 2 == 1))
                pendq = []

            # ================= Phase C: attention =================
            with (
                tc.tile_pool(name="zpA", bufs=6) as zpA,
                tc.tile_pool(name="zpB", bufs=6) as zpB,
                tc.tile_pool(name="nrm", bufs=2) as nrm,
                tc.tile_pool(name="nrm1", bufs=2) as nrm1,
                tc.tile_pool(name="dtp", bufs=4) as dtp,
                tc.tile_pool(name="bcp", bufs=3) as bcp,
                tc.tile_pool(name="ps_sc", bufs=2, space="PSUM") as ps_sc,
                tc.tile_pool(name="ps_y", bufs=1, space="PSUM") as ps_y,
            ):
                def emit_attnv(e):
                    """attnV for one (pair, s): each head's matmul is split
                    into two K=64 key-half tiles at row positions (0,0) and
                    (64,0) — concurrent in the PE array, and every LDWEIGHTS
                    targets row groups disjoint from the in-flight matmul so
                    weight loads hide. Halves accumulate into separate psum
                    banks (yaA+yaB summed during normalize)."""
                    zA, zB, ya, s, pr = e
                    yaA0, yaB0, yaA1, yaB1 = ya
                    st0, st1 = (s == 0), (s == NT - 1)
                    nc.tensor.matmul(
                        yaA0[:], vaug[0:64, s, 2 * pr, :], zA[0:64, :],
                        start=st0, stop=st1, tile_position=(0, 0))
                    nc.tensor.matmul(
                        yaB0[:], vaug[64:128, s, 2 * pr, :], zA[64:128, :],
                        start=st0, stop=st1, tile_position=(64, 0))
                    nc.tensor.matmul(
                        yaA1[:], vaug[0:64, s, 2 * pr + 1, :], zB[0:64, :],
                        start=st0, stop=st1, tile_position=(0, 0))
                    nc.tensor.matmul(
                        yaB1[:], vaug[64:128, s, 2 * pr + 1, :],
                        zB[64:128, :],
                        start=st0, stop=st1, tile_position=(64, 0))

                def make_norm(ya, pr, ch):
                    """normalize pair (ch, pr): 7 small closures staged
                    across the next pair's s-loop (slots >= 3, i.e. after
                    this pair's final attnV has been emitted). Evacuation
                    merges the key-half psum banks (copy + add: only one
                    PSUM operand per DVE instruction). Row 64 of the merged
                    tile is the softmax denominator; it is re-staged to a
                    partition-0 tile for the gpsimd broadcast."""
                    yab = ((ya[0], ya[1]), (ya[2], ya[3]))
                    st = {}
                    c0 = ch * CW

                    def evac(i):
                        a, b = yab[i]
                        yu = nrm.tile([65, CW], F32, tag=f"yu{i}",
                                      name=f"yu{i}_{ch}_{pr}")
                        nc.scalar.copy(yu[:], a[:])
                        nc.vector.tensor_tensor(
                            yu[:], yu[:], b[:], op=ALU.add)
                        st[f"yu{i}"] = yu

                    def bc(i):
                        # dt/bc rings are deeper than the other norm tiles:
                        # the Act(dt-copy) -> GpSimd(broadcast) WAR chain
                        # otherwise starves the Act exp queue at some pair
                        # boundaries (observed as 6.8us HAM half-clock
                        # dips). (A broadcast via SBUF->SBUF DMA with a
                        # 0-stride repeat is correct but ~250us slower:
                        # it lowers to per-row descriptors.)
                        dt = dtp.tile([1, CW], F32, tag=f"dt{i}",
                                      name=f"dt{i}_{ch}_{pr}")
                        nc.scalar.copy(dt[:], st[f"yu{i}"][64:65, :])
                        b = bcp.tile([64, CW], F32, tag=f"bc{i}",
                                     name=f"bc{i}_{ch}_{pr}")
                        nc.gpsimd.partition_broadcast(b[:], dt[:])
                        st[f"bc{i}"] = b

                    def recip():
                        for i in (0, 1):
                            rc = nrm1.tile([64, CW], F32, tag=f"bcr{i}",
                                           name=f"bcr{i}_{ch}_{pr}")
                            nc.vector.reciprocal_approx_fast(
                                rc[:], st[f"bc{i}"][:])
                            st[f"r{i}"] = rc

                    def mult(i):
                        # stays on DVE: gpsimd would work (all-SBUF) but
                        # switching its Q7 library away from
                        # PartitionBroadcast costs a multi-us reload per
                        # closure (measured +230us total)
                        rsl = slice(64 * i, 64 * i + 64)
                        nc.vector.tensor_tensor(
                            yTn[rsl, pr, c0:c0 + CW],
                            st[f"yu{i}"][0:64, :], st[f"r{i}"][:],
                            op=ALU.mult)

                    return [lambda: evac(0), lambda: evac(1),
                            lambda: bc(0), lambda: bc(1), recip,
                            lambda: mult(0), lambda: mult(1)]

                pend = []      # z tiles awaiting attnV (lag 3)
                norm_q = []    # previous pair's normalize closures
                NORM_SLOTS = (0, 1, 6, 11, 16, 21, 26)
                for ch in range(NCH):
                    for pr in range(PAIRS):
                        ya = tuple(
                            ps_y.tile([65, CW], F32, tag=f"ya{t}",
                                      name=f"ya{t}_{ch}_{pr}")
                            for t in ("A0", "B0", "A1", "B1"))
                        for s in range(NT):
                            # evacs at slots 0,1 (attnV'(0) at slot 3 needs
                            # the ya banks); the rest spread 5 slots apart
                            # so each ~700ns Act insertion amortizes against
                            # the ~140ns/step Act slack without delaying
                            # the exps enough to stall scores (HAM trips)
                            if norm_q and s == NORM_SLOTS[-len(norm_q)]:
                                norm_q.pop(0)()
                            ssl = slice(s * 128, (s + 1) * 128)
                            pscA = ps_sc.tile([128, CW], F32, tag="pscA")
                            pscB = ps_sc.tile([128, CW], F32, tag="pscB")
                            nc.tensor.matmul(
                                pscA[:], kT[0:64, pr, ssl],
                                qTc[ch][0:64, pr, :],
                                start=True, stop=True, tile_position=(0, 0))
                            nc.tensor.matmul(
                                pscB[:], kT[64:128, pr, ssl],
                                qTc[ch][64:128, pr, :],
                                start=True, stop=True, tile_position=(64, 0))
                            zA = zpA.tile([128, CW], BF16, tag="zA")
                            zB = zpB.tile([128, CW], BF16, tag="zB")
                            nc.scalar.activation(
                                zA[:], pscA[:], AF.Exp, scale=0.125)
                            nc.vector.tensor_scalar(
                                zB[:].bitcast(I16), pscB[:],
                                SCHRA, SCHRB, ALU.mult, ALU.add)
                            pend.append((zA, zB, ya, s, pr))
                            if len(pend) > 3:
                                emit_attnv(pend.pop(0))
                        # drain this pair's attnVs now (the last one waits
                        # ~0.3us on its exp) so the evacuations can run at
                        # the next pair's first slots — attnV'(0) then finds
                        # the ya banks free (no >2us stall, no HAM trip)
                        for e in pend:
                            emit_attnv(e)
                        pend = []
                        assert not norm_q
                        norm_q = make_norm(ya, pr, ch)
                for e in pend:
                    emit_attnv(e)
                pend = []
                for fn in norm_q:
                    fn()
                norm_q = []

            # ===== tail: out-projection for all t =====
            with (
                tc.tile_pool(name="ps_po", bufs=4, space="PSUM") as ps_po,
                tc.tile_pool(name="ostg", bufs=4) as ostg,
            ):
                # bridge the final normalize drain (DVE ~3us) so the PE
                # stays warm into the out-projection
                wdn = ps_po.tile([128, 512], F32, tag="po", name="wdn")
                for i in range(14):
                    nc.tensor.matmul(
                        wdn[:], kT[0:64, 0, 0:128], kT[0:64, 0, 0:512],
                        start=(i == 0), stop=(i == 13))
                for tt in range(NT):
                    tsl = slice(tt * 128, (tt + 1) * 128)
                    for oc in range(2):
                        po = ps_po.tile([128, 512], F32, tag="po")
                        for kp in range(4):
                            nc.tensor.matmul(
                                po[:], yTn[:, kp, tsl],
                                wo_sb[:, kp, oc * 512:(oc + 1) * 512],
                                start=(kp == 0), stop=(kp == 3))
                        ost = ostg.tile([128, 512], F32, tag="ost")
                        nc.vector.tensor_copy(ost[:], po[:])
                        nc.sync.dma_start(
                            out[tsl, oc * 512:(oc + 1) * 512], ost[:])

    nc.compile()
    return nc


_CACHE = {}


def _get_nc():
    if "nc" not in _CACHE:
        _CACHE["nc"] = build()
    return _CACHE["nc"]


def _prep_inputs(x, cos, sin, wq, wk, wv, wo):
    x = np.asarray(x, dtype=np.float32)
    cos = np.asarray(cos, dtype=np.float32).reshape(T, 32)
    sin = np.asarray(sin, dtype=np.float32).reshape(T, 32)
    wq = np.asarray(wq, dtype=np.float32)
    wk = np.asarray(wk, dtype=np.float32)
    wv = np.asarray(wv, dtype=np.float32)
    wo = np.asarray(wo, dtype=np.float32)

    cos2 = np.concatenate([cos, cos], axis=1)
    ss = np.concatenate([sin, -sin], axis=1)
    ident = np.eye(128, dtype=bf16)

    in_maps = []
    for c in range(8):
        b, hg = c // 2, c % 2
        rows = slice(hg * 512, (hg + 1) * 512)
        in_maps.append({
            "xT": np.ascontiguousarray(x[b].T).astype(bf16),
            "wqT": np.ascontiguousarray(wq[rows, :].T).astype(bf16),
            "wkT": np.ascontiguousarray(wk[rows, :].T).astype(bf16),
            "wvT": np.ascontiguousarray(wv[rows, :].T).astype(bf16),
            "woT": np.ascontiguousarray(wo[:, rows].T).astype(bf16),
            "cos2": cos2.astype(bf16),
            "ss": ss.astype(bf16),
            "ident": ident,
        })
    return in_maps


def _run(in_maps, trace=False):
    from concourse.bass_utils import run_bass_kernel_spmd

    nc = _get_nc()
    res = run_bass_kernel_spmd(nc, in_maps, core_ids=list(range(8)),
                               trace=trace)
    parts = [res.results[c]["out"] for c in range(8)]
    full = np.stack([parts[2 * b] + parts[2 * b + 1] for b in range(4)])
    return full.astype(np.float32), res


def kernel(x, cos, sin, wq, wk, wv, wo):
    in_maps = _prep_inputs(x, cos, sin, wq, wk, wv, wo)
    full, _ = _run(in_maps, trace=False)
    return full


# revision 49
# speedup vs baseline: 1.2275x; 1.0033x over previous
"""Distributed Trainium2 attention kernel (8 NeuronCores).

Sharding: 4-way data parallel over batch x 2-way tensor parallel over heads.
Core c handles batch c//2 and head-group c%2 (8 of 16 heads). Host sums the
two row-parallel out-proj partials per batch.

Structure (v2 — head-pair row-tiled attention):
- Phase A: per t-tile, all three projections (q,k,v) + rotary+rms for q and
  k + PE transposes into kT/qTc. PE ~5.7us/tile paces; Act does the copies/
  square/sqrt, DVE the rotary mults/reduce/reciprocal/scale.
- Phase C: heads processed in PAIRS. The two K=64 scores matmuls of a pair
  run CONCURRENTLY in the PE array via row tiling (tile_position (0,0) and
  (64,0)) — kT/qTc store the pair split at partition 64, so both tiles
  stream complementary partition ranges of the same SBUF columns (row
  tiling uses no extra XBUS). Per (pair, s-tile): 512 cy scores + 2x512 cy
  attnV = 1536 cy vs 4096 in the per-head serial schedule.
- attnV is ALSO split into K=64 key-half tiles at (0,0)/(64,0): with walrus'
  ldw-opt disabled every matmul self-loads weights, and a 64-row LDWEIGHTS
  hides under the opposite-half in-flight matmul; all-row (K=128) matmuls
  paid ~100ns exposed LDW per instruction.
- Softmax exp: one Act instruction for all of h0 (pscA) + one DVE Schraudolph
  for all of h1 (pscB). Separate psc tiles per head and separate z tiles/
  pools per writer are LOAD-BEARING: sharing a tile (or a psc read) between
  Act and DVE makes the tile-scheduler serialize the engines (its cost model
  underestimates PSUM-read ops ~2x), costing ~260us.
- attnV lags scores by 3 steps (z rings 8x), drained at each pair end so the
  next pair's evacuation closures (slots 0-6) free the single-buffered ya
  banks before attnV'(0) needs them (pair-boundary HAM trips otherwise).
- PSUM: pscA+pscB (2x2 banks double-buffered) + yaA0/yaB0/yaA1/yaB1 = 8.
- Softmax denominators ride as psum row 64 (ones column in vaug); per-pair
  normalize staged in small closures across the next pair's s-loop.
- Out-projection as a tail loop (PE has no idle in phase C); early po units
  only need early chunks' yTn so the final normalize drain hides under it.
"""
import sys
import os
from contextlib import ExitStack

if '/opt/trn_rl_repo' not in sys.path:
    sys.path.insert(0, '/opt/trn_rl_repo')

import numpy as np
import ml_dtypes

bf16 = ml_dtypes.bfloat16

T = 4096
D = 1024
HL = 8          # local heads per core
HD = 64
NT = T // 128   # 32 t-tiles
KT = D // 128   # 8 contraction tiles for projections
CW = 512        # chunk width (query columns per pair-step)
NCH = T // CW   # 8 chunks
PAIRS = 4       # head pairs per core
EPS = 1.1920928955078125e-07

EXPA = 512      # cols of the [128,1024] pair-psc on Act (true Exp = all of
                # h0); DVE Schraudolph covers all of h1. Separate z tiles so
                # Act and DVE never co-write one tile (no WAW serialization).
LN2 = 0.6931471805599453
# z = bitcast_bf16(int16(psc * SCHRA + SCHRB)) ~= exp(0.125 * psc) * const
SCHRA = 0.125 * (2.0 ** 23 / LN2) / 65536.0
SCHRB = (127.0 * 2.0 ** 23 - 485000.0) / 65536.0


def build():
    from concourse import bacc, tile, mybir

    BF16 = mybir.dt.bfloat16
    F32 = mybir.dt.float32
    I16 = mybir.dt.int16
    AF = mybir.ActivationFunctionType
    ALU = mybir.AluOpType
    AX = mybir.AxisListType

    nc = bacc.Bacc()
    xT = nc.declare_dram_parameter("xT", [D, T], BF16, isOutput=False)
    wqT = nc.declare_dram_parameter("wqT", [D, 512], BF16, isOutput=False)
    wkT = nc.declare_dram_parameter("wkT", [D, 512], BF16, isOutput=False)
    wvT = nc.declare_dram_parameter("wvT", [D, 512], BF16, isOutput=False)
    woT = nc.declare_dram_parameter("woT", [512, D], BF16, isOutput=False)
    cos2 = nc.declare_dram_parameter("cos2", [T, 64], BF16, isOutput=False)
    ss = nc.declare_dram_parameter("ss", [T, 64], BF16, isOutput=False)
    ident = nc.declare_dram_parameter("ident", [128, 128], BF16, isOutput=False)
    out = nc.declare_dram_parameter("out", [T, D], F32, isOutput=True)

    with tile.TileContext(nc) as tc:
        with tc.tile_pool(name="persist", bufs=1) as persist:
            qTc = [persist.tile([128, PAIRS, CW], BF16, tag=f"qT{c}",
                                name=f"qT{c}") for c in range(NCH)]
            kT = persist.tile([128, PAIRS, T], BF16, tag="kT")
            vaug = persist.tile([128, NT, HL, 65], BF16, tag="vaug")
            wo_sb = persist.tile([128, 4, D], BF16, tag="wo_sb")
            id_sb = persist.tile([128, 128], BF16, tag="id_sb")
            eps_t = persist.tile([128, 1], F32, tag="eps_t")
            yTn = persist.tile([128, PAIRS, T], BF16, tag="yTn")

            nc.vector.memset(vaug[:, :, :, 64:65], 1.0)
            nc.vector.memset(eps_t[:], EPS)

            # ================= Phase A: q/k/v for all t =================
            with ExitStack() as phaseA:
                wkv = phaseA.enter_context(tc.tile_pool(name="wkv", bufs=1))
                xcolp = phaseA.enter_context(
                    tc.tile_pool(name="xcolp", bufs=5))
                ascr = phaseA.enter_context(tc.tile_pool(name="ascr", bufs=3))
                asmall = phaseA.enter_context(
                    tc.tile_pool(name="asmall", bufs=3))
                ps_qkv = phaseA.enter_context(
                    tc.tile_pool(name="ps_qkv", bufs=2, space="PSUM"))
                ps_tr = phaseA.enter_context(
                    tc.tile_pool(name="ps_tr", bufs=2, space="PSUM"))

                def dma_xcol(t):
                    xcol = xcolp.tile([128, KT, 128], BF16, tag="xcol")
                    nc.sync.dma_start(
                        xcol[:],
                        xT[:, t * 128:(t + 1) * 128].rearrange(
                            "(k p) t -> p k t", p=128))
                    return xcol

                # DMA issue order = first-use order: tile 0 needs wk+xcol0
                # within ~5us, the rest can trail
                w_sb = {}
                for name, param in (("k", wkT), ("v", wvT), ("q", wqT)):
                    w_sb[name] = wkv.tile([128, KT, 512], BF16,
                                          tag=f"w{name}", name=f"w_{name}_sb")
                xq = [dma_xcol(0)]
                for ki in range(KT):
                    nc.sync.dma_start(
                        w_sb["k"][:, ki, :], wkT[ki * 128:(ki + 1) * 128, :])
                for ki in range(KT):
                    nc.sync.dma_start(
                        w_sb["v"][:, ki, :], wvT[ki * 128:(ki + 1) * 128, :])
                xq.append(dma_xcol(1))
                for ki in range(KT):
                    nc.sync.dma_start(
                        w_sb["q"][:, ki, :], wqT[ki * 128:(ki + 1) * 128, :])
                xq.append(dma_xcol(2))
                # cos/ss feed the rotary chains (DVE), which lag the
                # projections by design — their DMAs can land late
                cos_sb = wkv.tile([128, NT, 64], BF16, tag="cos_sb")
                ss_sb = wkv.tile([128, NT, 64], BF16, tag="ss_sb")
                nc.sync.dma_start(
                    cos_sb[:], cos2[:].rearrange("(t p) d -> p t d", p=128))
                nc.sync.dma_start(
                    ss_sb[:], ss[:].rearrange("(t p) d -> p t d", p=128))
                xq.append(dma_xcol(3))
                xq.append(dma_xcol(4))
                nc.sync.dma_start(id_sb[:], ident[:])

                # preload the GpSimd libraries (PartitionBroadcast + copy)
                # now so phase C's first use doesn't eat a Q7 reload
                pbsrc = asmall.tile([1, 8], F32, tag="pbsrc")
                nc.vector.memset(pbsrc[:], 1.0)
                pbdst = asmall.tile([128, 8], F32, tag="pbdst")
                nc.gpsimd.partition_broadcast(pbdst[:], pbsrc[:])

                def proj(xcol, name):
                    ps = ps_qkv.tile([128, 512], F32, tag=f"p{name}",
                                     name=f"ps_{name}")
                    for ki in range(KT):
                        nc.tensor.matmul(
                            ps[:], xcol[:, ki, :], w_sb[name][:, ki, :],
                            start=(ki == 0), stop=(ki == KT - 1))
                    return ps

                def rotary_rms(t, ps_q, store, tcol):
                    """rotary + rms-normalize one projected [128,512] tile.
                    Copies + square + sqrt on Act; mults/reduce/reciprocal/
                    scale on DVE. Returns the qn tile to transpose later."""
                    ctb = cos_sb[:, t, :].unsqueeze(1).broadcast_to(
                        [128, HL, 64])
                    stb = ss_sb[:, t, :].unsqueeze(1).broadcast_to(
                        [128, HL, 64])
                    qb = ascr.tile([128, 512], BF16, tag="qb")
                    nc.scalar.copy(qb[:], ps_q[:])
                    b3 = qb[:].rearrange("p (h u d) -> p h u d", h=HL, u=2)
                    qs = ascr.tile([128, 512], BF16, tag="qs")
                    qs3 = qs[:].rearrange("p (h u d) -> p h u d", h=HL, u=2)
                    nc.scalar.copy(qs3[:, :, 0, :], b3[:, :, 1, :])
                    nc.scalar.copy(qs3[:, :, 1, :], b3[:, :, 0, :])
                    t1 = ascr.tile([128, 512], BF16, tag="t1")
                    nc.vector.tensor_tensor(
                        t1[:].rearrange("p (h d) -> p h d", h=HL),
                        qb[:].rearrange("p (h d) -> p h d", h=HL),
                        ctb, op=ALU.mult)
                    r = ascr.tile([128, 512], BF16, tag="r")
                    nc.vector.tensor_tensor(
                        r[:].rearrange("p (h d) -> p h d", h=HL),
                        qs[:].rearrange("p (h d) -> p h d", h=HL),
                        stb, op=ALU.mult)
                    nc.vector.tensor_tensor(r[:], t1[:], r[:], op=ALU.add)
                    sq = ascr.tile([128, 512], BF16, tag="sq")
                    nc.scalar.square(sq[:], r[:])
                    ms8 = asmall.tile([128, HL], F32, tag="ms8")
                    nc.vector.tensor_reduce(
                        ms8[:], sq[:].rearrange("p (h d) -> p h d", h=HL),
                        axis=AX.X, op=ALU.add)
                    rms = asmall.tile([128, HL], F32, tag="rms")
                    nc.scalar.activation(
                        rms[:], ms8[:], AF.Sqrt, scale=1.0 / HD,
                        bias=eps_t[:])
                    rinv = asmall.tile([128, HL], F32, tag="rinv")
                    nc.vector.reciprocal(rinv[:], rms[:])
                    qn = ascr.tile([128, 512], BF16, tag="qn")
                    nc.vector.tensor_tensor(
                        qn[:].rearrange("p (h d) -> p h d", h=HL),
                        r[:].rearrange("p (h d) -> p h d", h=HL),
                        rinv[:].unsqueeze(2).broadcast_to([128, HL, 64]),
                        op=ALU.mult)
                    return (qn, store, tcol)

                def emit_tr(pend, on_act):
                    qn, store, tcol = pend
                    tp = ps_tr.tile([128, 4, 128], BF16, tag="tp")
                    for cb in range(4):
                        nc.tensor.transpose(
                            tp[:, cb, :], qn[:, cb * 128:(cb + 1) * 128],
                            id_sb[:])
                    cp = nc.scalar.copy if on_act else nc.vector.tensor_copy
                    cp(store[:, :, tcol:tcol + 128], tp[:])

                pendq = []
                for t in range(NT):
                    xcol = xq.pop(0)
                    if t + 5 < NT:
                        xq.append(dma_xcol(t + 5))
                    ps_k = proj(xcol, "k")
                    ps_v = proj(xcol, "v")
                    ps_q = proj(xcol, "q")
                    nc.scalar.copy(
                        vaug[:, t, :, 0:64],
                        ps_v[:].rearrange("p (h d) -> p h d", h=HL))
                    if len(pendq) >= 4:
                        emit_tr(pendq.pop(0), on_act=False)
                    pendq.append(rotary_rms(t, ps_k, kT, t * 128))
                    if len(pendq) >= 4:
                        emit_tr(pendq.pop(0), on_act=True)
                    pendq.append(
                        rotary_rms(t, ps_q, qTc[t // 4], (t % 4) * 128))
                # A->C bridge INSIDE phase A scope: Exp table load (ATL
                # ~2.7us on Act) + PE junk burst covering the last tiles'
                # DVE rotary drain, then the final transposes
                nc.sync.dma_start(
                    wo_sb[:], woT[:].rearrange("(k p) n -> p k n", p=128))
                dummy0 = asmall.tile([128, 1], F32, tag="dummy0")
                nc.scalar.activation(dummy0[:], eps_t[:], AF.Exp)
                # burst sized to the ~6us end-of-phase-A DVE rotary
                # backlog: PE chews junk at full clock while DVE drains,
                # so the drain transposes never leave a >2us PE idle
                # (the Exp table load was hoisted to kernel start by the
                # scheduler, so only the rotary drain needs covering)
                wup = ps_qkv.tile([128, 512], F32, tag="pk", name="wup")
                for i in range(30):
                    nc.tensor.matmul(
                        wup[:], kT[0:64, 0, 0:128], kT[0:64, 0, 0:512],
                        start=(i == 0), stop=(i == 29))
                for i, pend in enumerate(pendq):
                    emit_tr(pend, on_act=(i % 2 == 1))
                pendq = []

            # ================= Phase C: attention =================
            with (
                tc.tile_pool(name="zpA", bufs=6) as zpA,
                tc.tile_pool(name="zpB", bufs=6) as zpB,
                tc.tile_pool(name="nrm", bufs=2) as nrm,
                tc.tile_pool(name="nrm1", bufs=2) as nrm1,
                tc.tile_pool(name="dtp", bufs=4) as dtp,
                tc.tile_pool(name="bcp", bufs=3) as bcp,
                tc.tile_pool(name="ps_sc", bufs=2, space="PSUM") as ps_sc,
                tc.tile_pool(name="ps_y", bufs=1, space="PSUM") as ps_y,
            ):
                def emit_attnv(e):
                    """attnV for one (pair, s): each head's matmul is split
                    into two K=64 key-half tiles at row positions (0,0) and
                    (64,0) — concurrent in the PE array, and every LDWEIGHTS
                    targets row groups disjoint from the in-flight matmul so
                    weight loads hide. Halves accumulate into separate psum
                    banks (yaA+yaB summed during normalize)."""
                    zA, zB, ya, s, pr = e
                    yaA0, yaB0, yaA1, yaB1 = ya
                    st0, st1 = (s == 0), (s == NT - 1)
                    nc.tensor.matmul(
                        yaA0[:], vaug[0:64, s, 2 * pr, :], zA[0:64, :],
                        start=st0, stop=st1, tile_position=(0, 0))
                    nc.tensor.matmul(
                        yaB0[:], vaug[64:128, s, 2 * pr, :], zA[64:128, :],
                        start=st0, stop=st1, tile_position=(64, 0))
                    nc.tensor.matmul(
                        yaA1[:], vaug[0:64, s, 2 * pr + 1, :], zB[0:64, :],
                        start=st0, stop=st1, tile_position=(0, 0))
                    nc.tensor.matmul(
                        yaB1[:], vaug[64:128, s, 2 * pr + 1, :],
                        zB[64:128, :],
                        start=st0, stop=st1, tile_position=(64, 0))

                def make_norm(ya, pr, ch):
                    """normalize pair (ch, pr): 7 small closures staged
                    across the next pair's s-loop (slots >= 3, i.e. after
                    this pair's final attnV has been emitted). Evacuation
                    merges the key-half psum banks (copy + add: only one
                    PSUM operand per DVE instruction). Row 64 of the merged
                    tile is the softmax denominator; it is re-staged to a
                    partition-0 tile for the gpsimd broadcast."""
                    yab = ((ya[0], ya[1]), (ya[2], ya[3]))
                    st = {}
                    c0 = ch * CW

                    def evac(i):
                        a, b = yab[i]
                        yu = nrm.tile([65, CW], F32, tag=f"yu{i}",
                                      name=f"yu{i}_{ch}_{pr}")
                        nc.scalar.copy(yu[:], a[:])
                        nc.vector.tensor_tensor(
                            yu[:], yu[:], b[:], op=ALU.add)
                        st[f"yu{i}"] = yu

                    def bc(i):
                        # dt/bc rings are deeper than the other norm tiles:
                        # the Act(dt-copy) -> GpSimd(broadcast) WAR chain
                        # otherwise starves the Act exp queue at some pair
                        # boundaries (observed as 6.8us HAM half-clock
                        # dips). (A broadcast via SBUF->SBUF DMA with a
                        # 0-stride repeat is correct but ~250us slower:
                        # it lowers to per-row descriptors.)
                        dt = dtp.tile([1, CW], F32, tag=f"dt{i}",
                                      name=f"dt{i}_{ch}_{pr}")
                        nc.scalar.copy(dt[:], st[f"yu{i}"][64:65, :])
                        b = bcp.tile([64, CW], F32, tag=f"bc{i}",
                                     name=f"bc{i}_{ch}_{pr}")
                        nc.gpsimd.partition_broadcast(b[:], dt[:])
                        st[f"bc{i}"] = b

                    def recip():
                        for i in (0, 1):
                            rc = nrm1.tile([64, CW], F32, tag=f"bcr{i}",
                                           name=f"bcr{i}_{ch}_{pr}")
                            nc.vector.reciprocal_approx_fast(
                                rc[:], st[f"bc{i}"][:])
                            st[f"r{i}"] = rc

                    def mult(i):
                        # stays on DVE: gpsimd would work (all-SBUF) but
                        # switching its Q7 library away from
                        # PartitionBroadcast costs a multi-us reload per
                        # closure (measured +230us total)
                        rsl = slice(64 * i, 64 * i + 64)
                        nc.vector.tensor_tensor(
                            yTn[rsl, pr, c0:c0 + CW],
                            st[f"yu{i}"][0:64, :], st[f"r{i}"][:],
                            op=ALU.mult)

                    return [lambda: evac(0), lambda: evac(1),
                            lambda: bc(0), lambda: bc(1), recip,
                            lambda: mult(0), lambda: mult(1)]

                pend = []      # z tiles awaiting attnV (lag 3)
                norm_q = []    # previous pair's normalize closures
                NORM_SLOTS = (0, 1, 6, 11, 16, 21, 26)
                for ch in range(NCH):
                    for pr in range(PAIRS):
                        ya = tuple(
                            ps_y.tile([65, CW], F32, tag=f"ya{t}",
                                      name=f"ya{t}_{ch}_{pr}")
                            for t in ("A0", "B0", "A1", "B1"))
                        for s in range(NT):
                            # evacs at slots 0,1 (attnV'(0) at slot 3 needs
                            # the ya banks); the rest spread 5 slots apart
                            # so each ~700ns Act insertion amortizes against
                            # the ~140ns/step Act slack without delaying
                            # the exps enough to stall scores (HAM trips)
                            if norm_q and s == NORM_SLOTS[-len(norm_q)]:
                                norm_q.pop(0)()
                            ssl = slice(s * 128, (s + 1) * 128)
                            pscA = ps_sc.tile([128, CW], F32, tag="pscA")
                            pscB = ps_sc.tile([128, CW], F32, tag="pscB")
                            nc.tensor.matmul(
                                pscA[:], kT[0:64, pr, ssl],
                                qTc[ch][0:64, pr, :],
                                start=True, stop=True, tile_position=(0, 0))
                            nc.tensor.matmul(
                                pscB[:], kT[64:128, pr, ssl],
                                qTc[ch][64:128, pr, :],
                                start=True, stop=True, tile_position=(64, 0))
                            zA = zpA.tile([128, CW], BF16, tag="zA")
                            zB = zpB.tile([128, CW], BF16, tag="zB")
                            nc.scalar.activation(
                                zA[:], pscA[:], AF.Exp, scale=0.125)
                            nc.vector.tensor_scalar(
                                zB[:].bitcast(I16), pscB[:],
                                SCHRA, SCHRB, ALU.mult, ALU.add)
                            pend.append((zA, zB, ya, s, pr))
                            if len(pend) > 3:
                                emit_attnv(pend.pop(0))
                        # drain this pair's attnVs now (the last one waits
                        # ~0.3us on its exp) so the evacuations can run at
                        # the next pair's first slots — attnV'(0) then finds
                        # the ya banks free (no >2us stall, no HAM trip)
                        for e in pend:
                            emit_attnv(e)
                        pend = []
                        assert not norm_q
                        norm_q = make_norm(ya, pr, ch)
                for e in pend:
                    emit_attnv(e)
                pend = []
                for fn in norm_q:
                    fn()
                norm_q = []

            # ===== tail: out-projection for all t =====
            with (
                tc.tile_pool(name="ps_po", bufs=4, space="PSUM") as ps_po,
                tc.tile_pool(name="ostg", bufs=4) as ostg,
            ):
                # no bridge burst needed: po's first units read chunk 0's
                # yTn (written long ago), so the PE rolls straight from the
                # final attnV into the out-projection
                for tt in range(NT):
                    tsl = slice(tt * 128, (tt + 1) * 128)
                    for oc in range(2):
                        po = ps_po.tile([128, 512], F32, tag="po")
                        for kp in range(4):
                            nc.tensor.matmul(
                                po[:], yTn[:, kp, tsl],
                                wo_sb[:, kp, oc * 512:(oc + 1) * 512],
                                start=(kp == 0), stop=(kp == 3))
                        ost = ostg.tile([128, 512], F32, tag="ost")
                        nc.vector.tensor_copy(ost[:], po[:])
                        nc.sync.dma_start(
                            out[tsl, oc * 512:(oc + 1) * 512], ost[:])

    nc.compile()
    return nc


_CACHE = {}


def _get_nc():
    if "nc" not in _CACHE:
        _CACHE["nc"] = build()
    return _CACHE["nc"]


def _prep_inputs(x, cos, sin, wq, wk, wv, wo):
    x = np.asarray(x, dtype=np.float32)
    cos = np.asarray(cos, dtype=np.float32).reshape(T, 32)
    sin = np.asarray(sin, dtype=np.float32).reshape(T, 32)
    wq = np.asarray(wq, dtype=np.float32)
    wk = np.asarray(wk, dtype=np.float32)
    wv = np.asarray(wv, dtype=np.float32)
    wo = np.asarray(wo, dtype=np.float32)

    cos2 = np.concatenate([cos, cos], axis=1)
    ss = np.concatenate([sin, -sin], axis=1)
    ident = np.eye(128, dtype=bf16)

    in_maps = []
    for c in range(8):
        b, hg = c // 2, c % 2
        rows = slice(hg * 512, (hg + 1) * 512)
        in_maps.append({
            "xT": np.ascontiguousarray(x[b].T).astype(bf16),
            "wqT": np.ascontiguousarray(wq[rows, :].T).astype(bf16),
            "wkT": np.ascontiguousarray(wk[rows, :].T).astype(bf16),
            "wvT": np.ascontiguousarray(wv[rows, :].T).astype(bf16),
            "woT": np.ascontiguousarray(wo[:, rows].T).astype(bf16),
            "cos2": cos2.astype(bf16),
            "ss": ss.astype(bf16),
            "ident": ident,
        })
    return in_maps


def _run(in_maps, trace=False):
    from concourse.bass_utils import run_bass_kernel_spmd

    nc = _get_nc()
    res = run_bass_kernel_spmd(nc, in_maps, core_ids=list(range(8)),
                               trace=trace)
    parts = [res.results[c]["out"] for c in range(8)]
    full = np.stack([parts[2 * b] + parts[2 * b + 1] for b in range(4)])
    return full.astype(np.float32), res


def kernel(x, cos, sin, wq, wk, wv, wo):
    in_maps = _prep_inputs(x, cos, sin, wq, wk, wv, wo)
    full, _ = _run(in_maps, trace=False)
    return full


# revision 50
# speedup vs baseline: 1.2279x; 1.0003x over previous
"""Distributed Trainium2 attention kernel (8 NeuronCores).

Sharding: 4-way data parallel over batch x 2-way tensor parallel over heads.
Core c handles batch c//2 and head-group c%2 (8 of 16 heads). Host sums the
two row-parallel out-proj partials per batch.

Structure (v2 — head-pair row-tiled attention):
- Phase A: per t-tile, all three projections (q,k,v) + rotary+rms for q and
  k + PE transposes into kT/qTc. PE ~5.7us/tile paces; Act does the copies/
  square/sqrt, DVE the rotary mults/reduce/reciprocal/scale.
- Phase C: heads processed in PAIRS. The two K=64 scores matmuls of a pair
  run CONCURRENTLY in the PE array via row tiling (tile_position (0,0) and
  (64,0)) — kT/qTc store the pair split at partition 64, so both tiles
  stream complementary partition ranges of the same SBUF columns (row
  tiling uses no extra XBUS). Per (pair, s-tile): 512 cy scores + 2x512 cy
  attnV = 1536 cy vs 4096 in the per-head serial schedule.
- attnV is ALSO split into K=64 key-half tiles at (0,0)/(64,0): with walrus'
  ldw-opt disabled every matmul self-loads weights, and a 64-row LDWEIGHTS
  hides under the opposite-half in-flight matmul; all-row (K=128) matmuls
  paid ~100ns exposed LDW per instruction.
- Softmax exp: one Act instruction for all of h0 (pscA) + one DVE Schraudolph
  for all of h1 (pscB). Separate psc tiles per head and separate z tiles/
  pools per writer are LOAD-BEARING: sharing a tile (or a psc read) between
  Act and DVE makes the tile-scheduler serialize the engines (its cost model
  underestimates PSUM-read ops ~2x), costing ~260us.
- attnV lags scores by 3 steps (z rings 8x), drained at each pair end so the
  next pair's evacuation closures (slots 0-6) free the single-buffered ya
  banks before attnV'(0) needs them (pair-boundary HAM trips otherwise).
- PSUM: pscA+pscB (2x2 banks double-buffered) + yaA0/yaB0/yaA1/yaB1 = 8.
- Softmax denominators ride as psum row 64 (ones column in vaug); per-pair
  normalize staged in small closures across the next pair's s-loop.
- Out-projection as a tail loop (PE has no idle in phase C); early po units
  only need early chunks' yTn so the final normalize drain hides under it.
"""
import sys
import os
from contextlib import ExitStack

if '/opt/trn_rl_repo' not in sys.path:
    sys.path.insert(0, '/opt/trn_rl_repo')

import numpy as np
import ml_dtypes

bf16 = ml_dtypes.bfloat16

T = 4096
D = 1024
HL = 8          # local heads per core
HD = 64
NT = T // 128   # 32 t-tiles
KT = D // 128   # 8 contraction tiles for projections
CW = 512        # chunk width (query columns per pair-step)
NCH = T // CW   # 8 chunks
PAIRS = 4       # head pairs per core
EPS = 1.1920928955078125e-07

EXPA = 512      # cols of the [128,1024] pair-psc on Act (true Exp = all of
                # h0); DVE Schraudolph covers all of h1. Separate z tiles so
                # Act and DVE never co-write one tile (no WAW serialization).
LN2 = 0.6931471805599453
# z = bitcast_bf16(int16(psc * SCHRA + SCHRB)) ~= exp(0.125 * psc) * const
SCHRA = 0.125 * (2.0 ** 23 / LN2) / 65536.0
SCHRB = (127.0 * 2.0 ** 23 - 485000.0) / 65536.0


def build():
    from concourse import bacc, tile, mybir

    BF16 = mybir.dt.bfloat16
    F32 = mybir.dt.float32
    I16 = mybir.dt.int16
    AF = mybir.ActivationFunctionType
    ALU = mybir.AluOpType
    AX = mybir.AxisListType

    nc = bacc.Bacc()
    xT = nc.declare_dram_parameter("xT", [D, T], BF16, isOutput=False)
    wqT = nc.declare_dram_parameter("wqT", [D, 512], BF16, isOutput=False)
    wkT = nc.declare_dram_parameter("wkT", [D, 512], BF16, isOutput=False)
    wvT = nc.declare_dram_parameter("wvT", [D, 512], BF16, isOutput=False)
    woT = nc.declare_dram_parameter("woT", [512, D], BF16, isOutput=False)
    cos2 = nc.declare_dram_parameter("cos2", [T, 64], BF16, isOutput=False)
    ss = nc.declare_dram_parameter("ss", [T, 64], BF16, isOutput=False)
    ident = nc.declare_dram_parameter("ident", [128, 128], BF16, isOutput=False)
    out = nc.declare_dram_parameter("out", [T, D], F32, isOutput=True)

    with tile.TileContext(nc) as tc:
        with tc.tile_pool(name="persist", bufs=1) as persist:
            qTc = [persist.tile([128, PAIRS, CW], BF16, tag=f"qT{c}",
                                name=f"qT{c}") for c in range(NCH)]
            kT = persist.tile([128, PAIRS, T], BF16, tag="kT")
            vaug = persist.tile([128, NT, HL, 65], BF16, tag="vaug")
            wo_sb = persist.tile([128, 4, D], BF16, tag="wo_sb")
            id_sb = persist.tile([128, 128], BF16, tag="id_sb")
            eps_t = persist.tile([128, 1], F32, tag="eps_t")
            yTn = persist.tile([128, PAIRS, T], BF16, tag="yTn")

            nc.vector.memset(vaug[:, :, :, 64:65], 1.0)
            nc.vector.memset(eps_t[:], EPS)

            # ================= Phase A: q/k/v for all t =================
            with ExitStack() as phaseA:
                wkv = phaseA.enter_context(tc.tile_pool(name="wkv", bufs=1))
                xcolp = phaseA.enter_context(
                    tc.tile_pool(name="xcolp", bufs=5))
                ascr = phaseA.enter_context(tc.tile_pool(name="ascr", bufs=3))
                asmall = phaseA.enter_context(
                    tc.tile_pool(name="asmall", bufs=3))
                ps_qkv = phaseA.enter_context(
                    tc.tile_pool(name="ps_qkv", bufs=2, space="PSUM"))
                ps_tr = phaseA.enter_context(
                    tc.tile_pool(name="ps_tr", bufs=2, space="PSUM"))

                def dma_xcol(t):
                    xcol = xcolp.tile([128, KT, 128], BF16, tag="xcol")
                    nc.sync.dma_start(
                        xcol[:],
                        xT[:, t * 128:(t + 1) * 128].rearrange(
                            "(k p) t -> p k t", p=128))
                    return xcol

                # DMA issue order = first-use order: tile 0 needs wk+xcol0
                # within ~5us, the rest can trail
                w_sb = {}
                for name, param in (("k", wkT), ("v", wvT), ("q", wqT)):
                    w_sb[name] = wkv.tile([128, KT, 512], BF16,
                                          tag=f"w{name}", name=f"w_{name}_sb")
                xq = [dma_xcol(0)]
                for ki in range(KT):
                    nc.sync.dma_start(
                        w_sb["k"][:, ki, :], wkT[ki * 128:(ki + 1) * 128, :])
                for ki in range(KT):
                    nc.sync.dma_start(
                        w_sb["v"][:, ki, :], wvT[ki * 128:(ki + 1) * 128, :])
                xq.append(dma_xcol(1))
                for ki in range(KT):
                    nc.sync.dma_start(
                        w_sb["q"][:, ki, :], wqT[ki * 128:(ki + 1) * 128, :])
                xq.append(dma_xcol(2))
                # cos/ss feed the rotary chains (DVE), which lag the
                # projections by design — their DMAs can land late
                cos_sb = wkv.tile([128, NT, 64], BF16, tag="cos_sb")
                ss_sb = wkv.tile([128, NT, 64], BF16, tag="ss_sb")
                nc.sync.dma_start(
                    cos_sb[:], cos2[:].rearrange("(t p) d -> p t d", p=128))
                nc.sync.dma_start(
                    ss_sb[:], ss[:].rearrange("(t p) d -> p t d", p=128))
                xq.append(dma_xcol(3))
                xq.append(dma_xcol(4))
                nc.sync.dma_start(id_sb[:], ident[:])

                # preload the GpSimd libraries (PartitionBroadcast + copy)
                # now so phase C's first use doesn't eat a Q7 reload
                pbsrc = asmall.tile([1, 8], F32, tag="pbsrc")
                nc.vector.memset(pbsrc[:], 1.0)
                pbdst = asmall.tile([128, 8], F32, tag="pbdst")
                nc.gpsimd.partition_broadcast(pbdst[:], pbsrc[:])

                def proj(xcol, name):
                    ps = ps_qkv.tile([128, 512], F32, tag=f"p{name}",
                                     name=f"ps_{name}")
                    for ki in range(KT):
                        nc.tensor.matmul(
                            ps[:], xcol[:, ki, :], w_sb[name][:, ki, :],
                            start=(ki == 0), stop=(ki == KT - 1))
                    return ps

                def rotary_rms(t, ps_q, store, tcol):
                    """rotary + rms-normalize one projected [128,512] tile.
                    Copies + square + sqrt on Act; mults/reduce/reciprocal/
                    scale on DVE. Returns the qn tile to transpose later."""
                    ctb = cos_sb[:, t, :].unsqueeze(1).broadcast_to(
                        [128, HL, 64])
                    stb = ss_sb[:, t, :].unsqueeze(1).broadcast_to(
                        [128, HL, 64])
                    qb = ascr.tile([128, 512], BF16, tag="qb")
                    nc.scalar.copy(qb[:], ps_q[:])
                    b3 = qb[:].rearrange("p (h u d) -> p h u d", h=HL, u=2)
                    qs = ascr.tile([128, 512], BF16, tag="qs")
                    qs3 = qs[:].rearrange("p (h u d) -> p h u d", h=HL, u=2)
                    nc.scalar.copy(qs3[:, :, 0, :], b3[:, :, 1, :])
                    nc.scalar.copy(qs3[:, :, 1, :], b3[:, :, 0, :])
                    t1 = ascr.tile([128, 512], BF16, tag="t1")
                    nc.vector.tensor_tensor(
                        t1[:].rearrange("p (h d) -> p h d", h=HL),
                        qb[:].rearrange("p (h d) -> p h d", h=HL),
                        ctb, op=ALU.mult)
                    r = ascr.tile([128, 512], BF16, tag="r")
                    nc.vector.tensor_tensor(
                        r[:].rearrange("p (h d) -> p h d", h=HL),
                        qs[:].rearrange("p (h d) -> p h d", h=HL),
                        stb, op=ALU.mult)
                    nc.vector.tensor_tensor(r[:], t1[:], r[:], op=ALU.add)
                    sq = ascr.tile([128, 512], BF16, tag="sq")
                    nc.scalar.square(sq[:], r[:])
                    ms8 = asmall.tile([128, HL], F32, tag="ms8")
                    nc.vector.tensor_reduce(
                        ms8[:], sq[:].rearrange("p (h d) -> p h d", h=HL),
                        axis=AX.X, op=ALU.add)
                    rms = asmall.tile([128, HL], F32, tag="rms")
                    nc.scalar.activation(
                        rms[:], ms8[:], AF.Sqrt, scale=1.0 / HD,
                        bias=eps_t[:])
                    rinv = asmall.tile([128, HL], F32, tag="rinv")
                    nc.vector.reciprocal(rinv[:], rms[:])
                    qn = ascr.tile([128, 512], BF16, tag="qn")
                    nc.vector.tensor_tensor(
                        qn[:].rearrange("p (h d) -> p h d", h=HL),
                        r[:].rearrange("p (h d) -> p h d", h=HL),
                        rinv[:].unsqueeze(2).broadcast_to([128, HL, 64]),
                        op=ALU.mult)
                    return (qn, store, tcol)

                def emit_tr(pend, on_act):
                    qn, store, tcol = pend
                    tp = ps_tr.tile([128, 4, 128], BF16, tag="tp")
                    for cb in range(4):
                        nc.tensor.transpose(
                            tp[:, cb, :], qn[:, cb * 128:(cb + 1) * 128],
                            id_sb[:])
                    cp = nc.scalar.copy if on_act else nc.vector.tensor_copy
                    cp(store[:, :, tcol:tcol + 128], tp[:])

                pendq = []
                for t in range(NT):
                    xcol = xq.pop(0)
                    if t + 5 < NT:
                        xq.append(dma_xcol(t + 5))
                    ps_k = proj(xcol, "k")
                    ps_v = proj(xcol, "v")
                    ps_q = proj(xcol, "q")
                    nc.scalar.copy(
                        vaug[:, t, :, 0:64],
                        ps_v[:].rearrange("p (h d) -> p h d", h=HL))
                    if len(pendq) >= 4:
                        emit_tr(pendq.pop(0), on_act=False)
                    pendq.append(rotary_rms(t, ps_k, kT, t * 128))
                    if len(pendq) >= 4:
                        emit_tr(pendq.pop(0), on_act=True)
                    pendq.append(
                        rotary_rms(t, ps_q, qTc[t // 4], (t % 4) * 128))
                # A->C bridge INSIDE phase A scope: Exp table load (ATL
                # ~2.7us on Act) + PE junk burst covering the last tiles'
                # DVE rotary drain, then the final transposes
                nc.sync.dma_start(
                    wo_sb[:], woT[:].rearrange("(k p) n -> p k n", p=128))
                dummy0 = asmall.tile([128, 1], F32, tag="dummy0")
                nc.scalar.activation(dummy0[:], eps_t[:], AF.Exp)
                # burst sized to the ~6us end-of-phase-A DVE rotary
                # backlog: PE chews junk at full clock while DVE drains,
                # so the drain transposes never leave a >2us PE idle
                # (the Exp table load was hoisted to kernel start by the
                # scheduler, so only the rotary drain needs covering)
                wup = ps_qkv.tile([128, 512], F32, tag="pk", name="wup")
                for i in range(42):
                    nc.tensor.matmul(
                        wup[:], kT[0:64, 0, 0:128], kT[0:64, 0, 0:512],
                        start=(i == 0), stop=(i == 41))
                for i, pend in enumerate(pendq):
                    emit_tr(pend, on_act=(i % 2 == 1))
                pendq = []

            # ================= Phase C: attention =================
            with (
                tc.tile_pool(name="zpA", bufs=6) as zpA,
                tc.tile_pool(name="zpB", bufs=6) as zpB,
                tc.tile_pool(name="nrm", bufs=2) as nrm,
                tc.tile_pool(name="nrm1", bufs=2) as nrm1,
                tc.tile_pool(name="dtp", bufs=4) as dtp,
                tc.tile_pool(name="bcp", bufs=3) as bcp,
                tc.tile_pool(name="ps_sc", bufs=2, space="PSUM") as ps_sc,
                tc.tile_pool(name="ps_y", bufs=1, space="PSUM") as ps_y,
            ):
                def emit_attnv(e):
                    """attnV for one (pair, s): each head's matmul is split
                    into two K=64 key-half tiles at row positions (0,0) and
                    (64,0) — concurrent in the PE array, and every LDWEIGHTS
                    targets row groups disjoint from the in-flight matmul so
                    weight loads hide. Halves accumulate into separate psum
                    banks (yaA+yaB summed during normalize)."""
                    zA, zB, ya, s, pr = e
                    yaA0, yaB0, yaA1, yaB1 = ya
                    st0, st1 = (s == 0), (s == NT - 1)
                    nc.tensor.matmul(
                        yaA0[:], vaug[0:64, s, 2 * pr, :], zA[0:64, :],
                        start=st0, stop=st1, tile_position=(0, 0))
                    nc.tensor.matmul(
                        yaB0[:], vaug[64:128, s, 2 * pr, :], zA[64:128, :],
                        start=st0, stop=st1, tile_position=(64, 0))
                    nc.tensor.matmul(
                        yaA1[:], vaug[0:64, s, 2 * pr + 1, :], zB[0:64, :],
                        start=st0, stop=st1, tile_position=(0, 0))
                    nc.tensor.matmul(
                        yaB1[:], vaug[64:128, s, 2 * pr + 1, :],
                        zB[64:128, :],
                        start=st0, stop=st1, tile_position=(64, 0))

                def make_norm(ya, pr, ch):
                    """normalize pair (ch, pr): 7 small closures staged
                    across the next pair's s-loop (slots >= 3, i.e. after
                    this pair's final attnV has been emitted). Evacuation
                    merges the key-half psum banks (copy + add: only one
                    PSUM operand per DVE instruction). Row 64 of the merged
                    tile is the softmax denominator; it is re-staged to a
                    partition-0 tile for the gpsimd broadcast."""
                    yab = ((ya[0], ya[1]), (ya[2], ya[3]))
                    st = {}
                    c0 = ch * CW

                    def evac(i):
                        a, b = yab[i]
                        yu = nrm.tile([65, CW], F32, tag=f"yu{i}",
                                      name=f"yu{i}_{ch}_{pr}")
                        nc.scalar.copy(yu[:], a[:])
                        nc.vector.tensor_tensor(
                            yu[:], yu[:], b[:], op=ALU.add)
                        st[f"yu{i}"] = yu

                    def bc(i):
                        # dt/bc rings are deeper than the other norm tiles:
                        # the Act(dt-copy) -> GpSimd(broadcast) WAR chain
                        # otherwise starves the Act exp queue at some pair
                        # boundaries (observed as 6.8us HAM half-clock
                        # dips). (A broadcast via SBUF->SBUF DMA with a
                        # 0-stride repeat is correct but ~250us slower:
                        # it lowers to per-row descriptors.)
                        dt = dtp.tile([1, CW], F32, tag=f"dt{i}",
                                      name=f"dt{i}_{ch}_{pr}")
                        nc.scalar.copy(dt[:], st[f"yu{i}"][64:65, :])
                        b = bcp.tile([64, CW], F32, tag=f"bc{i}",
                                     name=f"bc{i}_{ch}_{pr}")
                        nc.gpsimd.partition_broadcast(b[:], dt[:])
                        st[f"bc{i}"] = b

                    def recip():
                        for i in (0, 1):
                            rc = nrm1.tile([64, CW], F32, tag=f"bcr{i}",
                                           name=f"bcr{i}_{ch}_{pr}")
                            nc.vector.reciprocal_approx_fast(
                                rc[:], st[f"bc{i}"][:])
                            st[f"r{i}"] = rc

                    def mult(i):
                        # stays on DVE: gpsimd would work (all-SBUF) but
                        # switching its Q7 library away from
                        # PartitionBroadcast costs a multi-us reload per
                        # closure (measured +230us total)
                        rsl = slice(64 * i, 64 * i + 64)
                        nc.vector.tensor_tensor(
                            yTn[rsl, pr, c0:c0 + CW],
                            st[f"yu{i}"][0:64, :], st[f"r{i}"][:],
                            op=ALU.mult)

                    return [lambda: evac(0), lambda: evac(1),
                            lambda: bc(0), lambda: bc(1), recip,
                            lambda: mult(0), lambda: mult(1)]

                pend = []      # z tiles awaiting attnV (lag 3)
                norm_q = []    # previous pair's normalize closures
                NORM_SLOTS = (0, 1, 6, 11, 16, 21, 26)
                for ch in range(NCH):
                    for pr in range(PAIRS):
                        ya = tuple(
                            ps_y.tile([65, CW], F32, tag=f"ya{t}",
                                      name=f"ya{t}_{ch}_{pr}")
                            for t in ("A0", "B0", "A1", "B1"))
                        for s in range(NT):
                            # evacs at slots 0,1 (attnV'(0) at slot 3 needs
                            # the ya banks); the rest spread 5 slots apart
                            # so each ~700ns Act insertion amortizes against
                            # the ~140ns/step Act slack without delaying
                            # the exps enough to stall scores (HAM trips)
                            if norm_q and s == NORM_SLOTS[-len(norm_q)]:
                                norm_q.pop(0)()
                            ssl = slice(s * 128, (s + 1) * 128)
                            pscA = ps_sc.tile([128, CW], F32, tag="pscA")
                            pscB = ps_sc.tile([128, CW], F32, tag="pscB")
                            nc.tensor.matmul(
                                pscA[:], kT[0:64, pr, ssl],
                                qTc[ch][0:64, pr, :],
                                start=True, stop=True, tile_position=(0, 0))
                            nc.tensor.matmul(
                                pscB[:], kT[64:128, pr, ssl],
                                qTc[ch][64:128, pr, :],
                                start=True, stop=True, tile_position=(64, 0))
                            zA = zpA.tile([128, CW], BF16, tag="zA")
                            zB = zpB.tile([128, CW], BF16, tag="zB")
                            nc.scalar.activation(
                                zA[:], pscA[:], AF.Exp, scale=0.125)
                            nc.vector.tensor_scalar(
                                zB[:].bitcast(I16), pscB[:],
                                SCHRA, SCHRB, ALU.mult, ALU.add)
                            pend.append((zA, zB, ya, s, pr))
                            if len(pend) > 3:
                                emit_attnv(pend.pop(0))
                        # drain this pair's attnVs now (the last one waits
                        # ~0.3us on its exp) so the evacuations can run at
                        # the next pair's first slots — attnV'(0) then finds
                        # the ya banks free (no >2us stall, no HAM trip)
                        for e in pend:
                            emit_attnv(e)
                        pend = []
                        assert not norm_q
                        norm_q = make_norm(ya, pr, ch)
                for e in pend:
                    emit_attnv(e)
                pend = []
                for fn in norm_q:
                    fn()
                norm_q = []

            # ===== tail: out-projection for all t =====
            with (
                tc.tile_pool(name="ps_po", bufs=4, space="PSUM") as ps_po,
                tc.tile_pool(name="ostg", bufs=4) as ostg,
            ):
                # no bridge burst needed: po's first units read chunk 0's
                # yTn (written long ago), so the PE rolls straight from the
                # final attnV into the out-projection
                for tt in range(NT):
                    tsl = slice(tt * 128, (tt + 1) * 128)
                    for oc in range(2):
                        po = ps_po.tile([128, 512], F32, tag="po")
                        for kp in range(4):
                            nc.tensor.matmul(
                                po[:], yTn[:, kp, tsl],
                                wo_sb[:, kp, oc * 512:(oc + 1) * 512],
                                start=(kp == 0), stop=(kp == 3))
                        ost = ostg.tile([128, 512], F32, tag="ost")
                        nc.vector.tensor_copy(ost[:], po[:])
                        nc.sync.dma_start(
                            out[tsl, oc * 512:(oc + 1) * 512], ost[:])

    nc.compile()
    return nc


_CACHE = {}


def _get_nc():
    if "nc" not in _CACHE:
        _CACHE["nc"] = build()
    return _CACHE["nc"]


def _prep_inputs(x, cos, sin, wq, wk, wv, wo):
    x = np.asarray(x, dtype=np.float32)
    cos = np.asarray(cos, dtype=np.float32).reshape(T, 32)
    sin = np.asarray(sin, dtype=np.float32).reshape(T, 32)
    wq = np.asarray(wq, dtype=np.float32)
    wk = np.asarray(wk, dtype=np.float32)
    wv = np.asarray(wv, dtype=np.float32)
    wo = np.asarray(wo, dtype=np.float32)

    cos2 = np.concatenate([cos, cos], axis=1)
    ss = np.concatenate([sin, -sin], axis=1)
    ident = np.eye(128, dtype=bf16)

    in_maps = []
    for c in range(8):
        b, hg = c // 2, c % 2
        rows = slice(hg * 512, (hg + 1) * 512)
        in_maps.append({
            "xT": np.ascontiguousarray(x[b].T).astype(bf16),
            "wqT": np.ascontiguousarray(wq[rows, :].T).astype(bf16),
            "wkT": np.ascontiguousarray(wk[rows, :].T).astype(bf16),
            "wvT": np.ascontiguousarray(wv[rows, :].T).astype(bf16),
            "woT": np.ascontiguousarray(wo[:, rows].T).astype(bf16),
            "cos2": cos2.astype(bf16),
            "ss": ss.astype(bf16),
            "ident": ident,
        })
    return in_maps


def _run(in_maps, trace=False):
    from concourse.bass_utils import run_bass_kernel_spmd

    nc = _get_nc()
    res = run_bass_kernel_spmd(nc, in_maps, core_ids=list(range(8)),
                               trace=trace)
    parts = [res.results[c]["out"] for c in range(8)]
    full = np.stack([parts[2 * b] + parts[2 * b + 1] for b in range(4)])
    return full.astype(np.float32), res


def kernel(x, cos, sin, wq, wk, wv, wo):
    in_maps = _prep_inputs(x, cos, sin, wq, wk, wv, wo)
    full, _ = _run(in_maps, trace=False)
    return full
